# revision 15
# baseline (speedup 1.0000x reference)
"""Causal multi-head attention (B=4, S=2048, D=1024, H=16, HD=64) on 8 NeuronCores.

Sharding: core c handles batch b=c//2 and head-group hg=c%2 (8 heads each).
Each core computes out^T_partial = Wo_hg^T @ ctx_hg^T for its (b, hg); the host
sums the two head-group partials per batch, transposes, and adds the bias.

Heads are processed in pairs (2m, 2m+1). The K=64 score matmuls of a pair run
concurrently in the PE array via row tiling (tile_position (0,0)/(64,0)), the
pair's exp runs as one ScalarE instruction across both PSUM banks, and the
softmax denominators ride in a 65th V column whose reciprocal is taken straight
off the PSUM row at partition 64.
"""

import sys

for _p in ("/opt/trn_rl_repo",):
    if _p not in sys.path:
        sys.path.insert(0, _p)

import numpy as np
import ml_dtypes
from contextlib import ExitStack

import concourse.bacc as bacc
import concourse.tile as tile
from concourse import mybir
from concourse.bass_utils import run_bass_kernel_spmd

F32 = mybir.dt.float32
BF16 = mybir.dt.bfloat16
Exp = mybir.ActivationFunctionType.Exp
Copy = mybir.ActivationFunctionType.Copy
Mult = mybir.AluOpType.mult

B, S, D, H, HD = 4, 2048, 1024, 16, 64
NC = 8          # cores
HL = 8          # heads per core (head-group)
DH = HL * HD    # 512, per-core head dim
KT = D // 128   # 8 k-tiles over d_in
ST = S // 128   # 16 tiles over sequence
NB = S // 512   # 4 q-superblocks
SCALE = 1.0 / np.sqrt(HD)


def _build_nc():
    nc = bacc.Bacc("TRN2", target_bir_lowering=False)

    xT = nc.declare_dram_parameter("xT", [D, S], BF16, isOutput=False)
    wq = nc.declare_dram_parameter("wq", [D, DH], BF16, isOutput=False)
    wk = nc.declare_dram_parameter("wk", [D, DH], BF16, isOutput=False)
    wv = nc.declare_dram_parameter("wv", [D, DH], BF16, isOutput=False)
    wo = nc.declare_dram_parameter("wo", [DH, D], BF16, isOutput=False)
    tri = nc.declare_dram_parameter("tri", [128, 128], BF16, isOutput=False)
    outT = nc.declare_dram_parameter("outT", [D, S], F32, isOutput=True)

    with tile.TileContext(nc) as tc, ExitStack() as ctx:
        const_pool = ctx.enter_context(tc.tile_pool(name="const", bufs=1))
        xT_pool = ctx.enter_context(tc.tile_pool(name="xT", bufs=1))
        w_pool = ctx.enter_context(tc.tile_pool(name="w", bufs=1))
        qk_pool = ctx.enter_context(tc.tile_pool(name="qk", bufs=1))
        v_pool = ctx.enter_context(tc.tile_pool(name="v", bufs=1))
        ctxT_pool = ctx.enter_context(tc.tile_pool(name="ctxT", bufs=1))
        e_pool = ctx.enter_context(tc.tile_pool(name="e", bufs=4))
        r_pool = ctx.enter_context(tc.tile_pool(name="r", bufs=2))
        o_pool = ctx.enter_context(tc.tile_pool(name="o", bufs=2))
        ps_gen = ctx.enter_context(tc.tile_pool(name="ps_gen", bufs=1, space="PSUM"))
        ps_s = ctx.enter_context(tc.tile_pool(name="ps_s", bufs=2, space="PSUM"))
        ps_c = ctx.enter_context(tc.tile_pool(name="ps_c", bufs=1, space="PSUM"))

        # ---- constants ----
        trit = const_pool.tile([128, 128], BF16)
        nc.sync.dma_start(trit[:], tri[:])

        # ---- input DMAs: wv first (feeds emit_v + warmup), then x, q/k/o ----
        wvt = [w_pool.tile([128, DH], BF16, tag=f"wvt{_}", name=f"wvt{_}") for _ in range(KT)]
        for k in range(KT):
            nc.sync.dma_start(wvt[k][:], wv[128 * k : 128 * (k + 1), :])
        xt = [xT_pool.tile([128, S], BF16, tag=f"xt{_}", name=f"xt{_}") for _ in range(KT)]
        for k in range(KT):
            for h in (0, 1):
                nc.sync.dma_start(
                    xt[k][:, 1024 * h : 1024 * (h + 1)],
                    xT[128 * k : 128 * (k + 1), 1024 * h : 1024 * (h + 1)],
                )
        wqt = [w_pool.tile([128, DH], BF16, tag=f"wqt{_}", name=f"wqt{_}") for _ in range(KT)]
        wkt = [w_pool.tile([128, DH], BF16, tag=f"wkt{_}", name=f"wkt{_}") for _ in range(KT)]
        for k in range(KT):
            nc.sync.dma_start(wqt[k][:], wq[128 * k : 128 * (k + 1), :])
            nc.sync.dma_start(wkt[k][:], wk[128 * k : 128 * (k + 1), :])
        wot = [w_pool.tile([128, D], BF16, tag=f"wot{_}", name=f"wot{_}") for _ in range(DH // 128)]
        for k in range(DH // 128):
            nc.sync.dma_start(wot[k][:], wo[128 * k : 128 * (k + 1), :])

        # ---- HAM warm-up: junk matmuls gated on the first weight DMA so the
        # PE's clock gate is at 8/8 when real work starts ----
        junk = ps_gen.tile([128, 512], F32, tag="pgA", name="junk")
        for _ in range(24):
            nc.tensor.matmul(junk[:], wvt[0][:, 0:128], wvt[0][:], start=True, stop=True)

        # ---- V natural [S, DH] as 16 tiles [128, 8*65] (ones col per head) ----
        vt = [v_pool.tile([128, HL * (HD + 1)], BF16, tag=f"v{_}", name=f"v{_}") for _ in range(ST)]

        # Filler machinery: generators emit one small PE chunk per next();
        # attention j-loops drain them so projection work interleaves into
        # the exp-bound attention phase (the static scheduler won't do this
        # on its own — priority follows emission order).
        filler_q = []

        def drain(n):
            done = 0
            while filler_q and done < n:
                try:
                    next(filler_q[0])
                    done += 1
                except StopIteration:
                    filler_q.pop(0)

        def run_gen(g):
            for _ in g:
                pass

        def v_gen(st):
            nc.gpsimd.memset(vt[st].rearrange("p (h c) -> p h c", c=HD + 1)[:, :, HD], 1.0)
            pv = ps_gen.tile([128, 512], F32, tag=("pgA" if st % 2 == 0 else "pgB"), name="pv")
            for k in range(KT):
                nc.tensor.matmul(
                    pv[:], xt[k][:, 128 * st : 128 * (st + 1)], wvt[k][:],
                    start=(k == 0), stop=(k == KT - 1),
                )
                yield
            nc.vector.tensor_copy(
                vt[st].rearrange("p (h c) -> p h c", c=HD + 1)[:, :, 0:HD],
                pv.rearrange("p (h c) -> p h c", c=HD)[:],
            )

        # ---- q^T / k^T per head pair m: [128, S], heads 2m / 2m+1 in rows 0-63 / 64-127 ----
        qTt = [qk_pool.tile([128, S], BF16, tag=f"qT{_}", name=f"qT{_}") for _ in range(DH // 128)]
        kTt = [qk_pool.tile([128, S], BF16, tag=f"kT{_}", name=f"kT{_}") for _ in range(DH // 128)]

        def qk_gen(m, np_):
            for wt, dst in ((wqt, qTt), (wkt, kTt)):
                psA = ps_gen.tile([128, 512], F32, tag="pgA", name="psA")
                psB = ps_gen.tile([128, 512], F32, tag="pgB", name="psB")
                for k in range(KT):
                    lhsT = wt[k][:, 128 * m : 128 * (m + 1)]
                    for n, pst in ((2 * np_, psA), (2 * np_ + 1, psB)):
                        nc.tensor.matmul(
                            pst[:], lhsT, xt[k][:, 512 * n : 512 * (n + 1)],
                            start=(k == 0), stop=(k == KT - 1),
                        )
                    yield
                for n, pst in ((2 * np_, psA), (2 * np_ + 1, psB)):
                    nc.vector.tensor_copy(dst[m][:, 512 * n : 512 * (n + 1)], pst[:])
                yield

        # ---- attention for head pair m, one q-superblock I at a time ----
        ctxT = [ctxT_pool.tile([128, S], BF16, tag=f"ctxT{_}", name=f"ctxT{_}") for _ in range(DH // 128)]

        def attn_I(m, I, drain_n=1):
            vcA = slice(65 * (2 * m), 65 * (2 * m) + 65)
            vcB = slice(65 * (2 * m + 1), 65 * (2 * m + 1) + 65)
            if True:
                cpair = ps_c.tile([65, 1024], F32, tag="cp", name="cpair")
                jmax = 4 * I + 3

                def emit_ctx(j, e):
                    # ctx accumulation for one (I, j); diagonal blocks split so
                    # the mask only gates the 128-col part
                    diag = j >= 4 * I
                    lo = 128 * (j - 4 * I) if diag else 0
                    for vc, off in ((vcA, 0), (vcB, 512)):
                        if diag:
                            if lo + 128 < 512:
                                nc.tensor.matmul(
                                    cpair[:, off + lo + 128 : off + 512],
                                    vt[j][:, vc], e[:, off + lo + 128 : off + 512],
                                    start=(j == 0), stop=False, skip_group_check=True,
                                )
                            nc.tensor.matmul(
                                cpair[:, off + lo : off + lo + 128],
                                vt[j][:, vc], e[:, off + lo : off + lo + 128],
                                start=False, stop=(j == jmax), skip_group_check=True,
                            )
                        else:
                            nc.tensor.matmul(
                                cpair[:, off : off + 512],
                                vt[j][:, vc], e[:, off : off + 512],
                                start=(j == 0), stop=False, skip_group_check=True,
                            )

                prev = None  # (j, e) pending ctx emission
                for j in range(jmax + 1):
                    diag = j >= 4 * I
                    lo = 128 * (j - 4 * I) if diag else 0
                    qA = qTt[m][0:64, 512 * I + lo : 512 * (I + 1)]
                    qB = qTt[m][64:128, 512 * I + lo : 512 * (I + 1)]
                    sp = ps_s.tile([128, 1024], F32, tag="sp", name="sp")
                    nc.tensor.matmul(
                        sp[:, lo:512], kTt[m][0:64, 128 * j : 128 * (j + 1)], qA,
                        start=True, stop=True, tile_position=(0, 0),
                    )
                    nc.tensor.matmul(
                        sp[:, 512 + lo : 1024], kTt[m][64:128, 128 * j : 128 * (j + 1)], qB,
                        start=True, stop=True, tile_position=(64, 0),
                    )
                    e = e_pool.tile([128, 1024], BF16, tag="e", name="e")
                    # one flat exp; for diag j the [512:512+lo] garbage region is
                    # never read downstream
                    nc.scalar.activation(e[:, lo:1024], sp[:, lo:1024], Exp, scale=float(SCALE))
                    if diag:
                        for base in (lo, 512 + lo):
                            nc.vector.tensor_tensor(
                                e[:, base : base + 128], e[:, base : base + 128],
                                trit[:], Mult,
                            )
                    drain(drain_n)
                    if prev is not None:
                        emit_ctx(*prev)
                    prev = (j, e)
                emit_ctx(*prev)
                # normalize: den rows sit at partition 64 of each cpair bank.
                # Broadcast the bf16 den row down to 64 partitions via a K=1
                # matmul at partition 64, then take the reciprocal at part 0
                # (reciprocal_approx_fast only works at base partition 0).
                # Evacuate unnormalized ctx + den row quickly (releases cpair),
                # then normalize ctxT in place lazily — out-proj is the only
                # reader of ctxT and runs at the very end.
                qcols = slice(512 * I, 512 * (I + 1))
                nc.vector.tensor_copy(ctxT[m][0:64, qcols], cpair[0:64, 0:512])
                stB = r_pool.tile([64, 512], BF16, tag="stB", name="stB")
                nc.vector.tensor_copy(stB[:], cpair[0:64, 512:1024])
                nc.sync.dma_start(ctxT[m][64:128, qcols], stB[:])
                denf = r_pool.tile([65, 1024], F32, tag="denf", name="denf")
                nc.vector.tensor_copy(denf[64:65, :], cpair[64:65, :])
                den0 = r_pool.tile([1, 1024], F32, tag="den0", name="den0")
                nc.sync.dma_start(den0[0:1, :], denf[64:65, :])
                rec0 = r_pool.tile([1, 1024], F32, tag="rec0", name="rec0")
                nc.vector.reciprocal_approx_fast(rec0[0:1, :], den0[0:1, :])
                recs = r_pool.tile([128, 1024], F32, tag="recs", name="recs")
                nc.gpsimd.partition_broadcast(recs[:], rec0[0:1, :], channels=128)
                nc.vector.tensor_tensor(
                    ctxT[m][0:64, qcols], ctxT[m][0:64, qcols], recs[0:64, 0:512], Mult,
                )
                nc.vector.tensor_tensor(
                    ctxT[m][64:128, qcols], ctxT[m][64:128, qcols],
                    recs[64:128, 512:1024], Mult,
                )

        for st in range(4):
            run_gen(v_gen(st))
        run_gen(qk_gen(0, 0))
        filler_q.extend(v_gen(s) for s in range(4, 8))
        filler_q.append(qk_gen(0, 1))
        filler_q.extend(v_gen(s) for s in range(8, ST))
        filler_q.append(qk_gen(1, 0))
        filler_q.append(qk_gen(1, 1))
        for I_ in range(NB):
            attn_I(0, I_, drain_n=4)
        filler_q.append(qk_gen(2, 0))
        filler_q.append(qk_gen(2, 1))
        for I_ in range(NB):
            attn_I(1, I_, drain_n=2)
        filler_q.append(qk_gen(3, 0))
        filler_q.append(qk_gen(3, 1))
        for I_ in range(NB):
            attn_I(2, I_, drain_n=2)
        for I_ in range(NB):
            attn_I(3, I_, drain_n=1)
        drain(1000)

        # ---- out^T = Wo^T @ ctx^T  [D, S] ----
        for m in range(D // 128):
            ot = o_pool.tile([128, S], F32, tag="ot")
            for np_ in range(NB // 2):
                psA = ps_gen.tile([128, 512], F32, tag="pgA", name="poA")
                psB = ps_gen.tile([128, 512], F32, tag="pgB", name="poB")
                for k in range(DH // 128):
                    lhsT = wot[k][:, 128 * m : 128 * (m + 1)]
                    for n, pst in ((2 * np_, psA), (2 * np_ + 1, psB)):
                        nc.tensor.matmul(
                            pst[:], lhsT, ctxT[k][:, 512 * n : 512 * (n + 1)],
                            start=(k == 0), stop=(k == DH // 128 - 1),
                        )
                for n, pst in ((2 * np_, psA), (2 * np_ + 1, psB)):
                    nc.vector.tensor_copy(ot[:, 512 * n : 512 * (n + 1)], pst[:])
            nc.sync.dma_start(outT[128 * m : 128 * (m + 1), :], ot[:])

    nc.compile()
    return nc


_NC_CACHE = None


def kernel(x, Wq, Wk, Wv, Wo, bo):
    global _NC_CACHE
    if _NC_CACHE is None:
        _NC_CACHE = _build_nc()
    nc = _NC_CACHE

    bf = ml_dtypes.bfloat16
    tri = np.triu(np.ones((128, 128), dtype=np.float32)).astype(bf)
    in_maps = []
    for c in range(NC):
        b, hg = c // 2, c % 2
        cols = slice(DH * hg, DH * (hg + 1))
        in_maps.append(
            {
                "xT": np.ascontiguousarray(np.asarray(x)[b].T).astype(bf),
                "wq": np.asarray(Wq)[:, cols].astype(bf),
                "wk": np.asarray(Wk)[:, cols].astype(bf),
                "wv": np.asarray(Wv)[:, cols].astype(bf),
                "wo": np.asarray(Wo)[cols, :].astype(bf),
                "tri": tri,
            }
        )
    res = run_bass_kernel_spmd(nc, in_maps, core_ids=list(range(NC)))
    out = np.empty((B, S, D), dtype=np.float32)
    bo32 = np.asarray(bo, dtype=np.float32)
    for b in range(B):
        acc = res.results[2 * b]["outT"].astype(np.float32) + res.results[2 * b + 1][
            "outT"
        ].astype(np.float32)
        out[b] = acc.T + bo32
    return out
